# revision 30
# baseline (speedup 1.0000x reference)
"""Trainium2 Bass kernel for nn_AttentionModule: full-sequence self-attention.

Reference computation (all fp32):
    x = inputs @ W_proj + b_proj            # [B,4096,256]   (B=4, N=4096)
    q,k,v = x@W_q+b_q, x@W_k+b_k, x@W_v+b_v
    attn = softmax(q @ k^T)                 # [B,4096,4096]
    out  = gamma * (attn @ v) + x

Sharding: 8 cores = 4 batches x 2 query-halves. Core c handles batch
b=c//2, query rows h*2048..h*2048+2048 (h=c%2); keys/values span the
full 4096 sequence of its batch (sequence rotated host-side; attention
is invariant under the joint key/value permutation).

Host-side algebra (exact up to fp reassociation):
    scores contract in channel space (rank <= C_IN=128):
        s_{q,k} = y_q . x_k + r . x_k,  y_q = m_qk^T x_q,
        m_qk = (Wp Wq)(Wp Wk)^T,  r = (Wp Wk)(bp Wq + b_q)
    attnV is factored through X:  E @ V = (E^T-contracted X) @ (g Wp Wv),
    so no V matrix is ever materialized on device; gamma and the value/
    residual biases fold into the projection weight / x-path bias.

Device program per core, fp8-e4m3 DoubleRow matmuls (0.5 cyc/row, 2x
the f32r rate; DR contracts 2x128 via [P,2,free] operand pairing):
    scores   S^T [128k x 512q] = DR(in8 [64,2,128k], Y8 [64,2,512q]);
             two key blocks per [128,1024] psum pair-tile (channel
             pairs c = p + 64j, split host-side).
    exp      each pair-tile's four 128-query sub-blocks split between
             ACT (true exp -> fp8) and DVE (exact-int8 Schraudolph:
             e = bitcast_e4m3(round(s*8/ln2 + 56)), piecewise-linear
             exp, <4% rel err - harmless since the attention context is
             ~0.2% of the residual x).  One e-tile PER ENGINE per step:
             the tile scheduler chains same-tile writers, which would
             serialize the stripes.  GPSIMD cannot read PSUM on hw, so
             it only gets SBUF-side epilogue adds.
    attnV    factored: C_x [128ch x 512q] += DR(in_seq8 [128k,2,128ch],
             e-tile [128k,2,w]) over 16 key-pair steps -- half the PE
             work of E@V and no V-conversion copies.  Per ic epilogue:
             C_x -> bf16, project with w_pvb [128,256] (4 matmuls into
             a scores-pool psum), denominators via a burst of tiny DR
             ones-matmuls over the ic's retained e-tiles (e-pool is 26
             deep so they survive), then out = proj*recip(denom) + x_sb
             (fused DVE affine_then_add or ACT-mul + Pool-add), DMA out.

f32r (tf32-like) matmuls for accuracy-critical paths: Y and the
residual x = X W_p (query columns only - inT is [128,2048]); f32r
inputs are pre-rounded host-side and DMA'd directly.  PSUM budget
(16KB/partition, bank-quantized slots): scores 3 bufs x 2 banks + C_x
2 bufs x 1 bank; setup/projection/denominator psums share the 3-deep
scores rotation.  Software pipelining: S-pairs emitted `ahead`=5 steps
before their C_x consumers; input DMA split across HWDGE + SWDGE
queues, critical chunks (m2, inT ic0, in8 head) first.

Cost-model exec time: 73.0us/core (original baseline 124.4us).
Measured on trn2 (8 cores): rel err 3.0e-04 vs fp32 jax reference.
"""

import numpy as np
from contextlib import ExitStack

import concourse.bass as bass
import concourse.tile as tile
from concourse import bacc, mybir
from concourse.bass_utils import run_bass_kernel_spmd

B, SEQ, C_IN, F = 4, 4096, 128, 256
N_CORES = 8
QROWS = SEQ // 2              # queries per core
N_IC = 4                      # 512-query chunks
N_T2 = 16                     # key-pair steps per ic (256 keys each)
F32, F32R, FP8 = mybir.dt.float32, mybir.dt.float32r, mybir.dt.float8e4
I8 = mybir.dt.int8
DR = mybir.MatmulPerfMode.DoubleRow
EXP_A = 8.0 / float(np.log(2.0))
EXP_B = 56.0


def default_stripes():
    # Per-step isub ownership (n_act, n_dve, n_pool) out of 4 query
    # sub-blocks.  Separate tiles per engine avoid the scheduler's
    # same-tile writer chaining (which would serialize the stripes).
    # GPSIMD cannot read PSUM on hw, so n_pool stays 0.
    return [(2, 2, 0)] * 64


def default_vx_engines():
    # engines for the 16 v8 copies and 8 x copies (setup, front-loaded;
    # GPSIMD cannot read PSUM, so only act/dve are legal here)
    v = ["act", "dve"] * 8
    x = ["dve", "act"] * 4
    return v, x


def build_bass(stripes=None, s_bufs=3, e_bufs=26, use_bias=False,
               y8_eng="act", v_engines=None, x_engines=None,
               ep_engines=("dve", "actpool", "dve", "actpool"), ahead=5):
    if stripes is None:
        stripes = default_stripes()
    dv, dx = default_vx_engines()
    if v_engines is None:
        v_engines = dv
    if x_engines is None:
        x_engines = dx
    nc = bacc.Bacc("TRN2", target_bir_lowering=False, debug=False,
                   num_devices=N_CORES)
    d_inT = nc.dram_tensor("inT_r", [C_IN, QROWS], F32R, kind="ExternalInput").ap()
    d_in8 = nc.dram_tensor("in8", [64, 2 * SEQ], FP8, kind="ExternalInput").ap()
    d_m2 = nc.dram_tensor("m2", [C_IN, C_IN], F32R, kind="ExternalInput").ap()
    d_iseq = nc.dram_tensor("in_seq8", [128, SEQ], FP8, kind="ExternalInput").ap()
    d_wpv = nc.dram_tensor("w_pvb", [C_IN, F], mybir.dt.bfloat16,
                           kind="ExternalInput").ap()
    d_wp = nc.dram_tensor("w_p", [C_IN, F], F32R, kind="ExternalInput").ap()
    d_rb = nc.dram_tensor("r_bias", [C_IN, 2], F32, kind="ExternalInput").ap()
    d_bx = nc.dram_tensor("bias_x_bc", [128, F], F32, kind="ExternalInput").ap()
    d_out = nc.dram_tensor("out", [QROWS, F], F32, kind="ExternalOutput").ap()

    eng = {"act": nc.scalar, "dve": nc.vector, "pool": nc.gpsimd}

    def ecopy(which, dst, src_ap):
        if which == "act":
            nc.scalar.copy(dst, src_ap)
        else:
            eng[which].tensor_copy(dst, src_ap)

    with tile.TileContext(nc) as tc, ExitStack() as ctx:
        per = ctx.enter_context(tc.tile_pool(name="per", bufs=1))
        epool = ctx.enter_context(tc.tile_pool(name="epool", bufs=e_bufs))
        opool = ctx.enter_context(tc.tile_pool(name="opool", bufs=6))
        ps_s = ctx.enter_context(tc.tile_pool(name="ps_s", bufs=s_bufs,
                                              space="PSUM"))
        ps_c = ctx.enter_context(tc.tile_pool(name="ps_c", bufs=2,
                                              space="PSUM"))

        # ---- input DMA ---------------------------------------------------
        # Critical path to the first scores matmul: m2 + inT chunk 0
        # (-> Y8 ic0) and the first quarter of in8.  Order the sync queue
        # accordingly; everything else follows.
        m2 = per.tile([C_IN, C_IN], F32R, tag="m2")
        nc.sync.dma_start(m2[:], d_m2[:])
        inT = per.tile([C_IN, QROWS], F32R, tag="inT")
        in8 = per.tile([64, 2 * SEQ], FP8, tag="in8")
        iseq = per.tile([128, SEQ], FP8, tag="iseq")
        wpv = per.tile([C_IN, F], mybir.dt.bfloat16, tag="wpv")
        wp = per.tile([C_IN, F], F32R, tag="wp")
        # inT on the HWDGE queue; in8/weights in parallel on the SWDGE
        # queue (trigger cost lands on the otherwise-idle early Pool)
        nc.sync.dma_start(inT[:, bass.ts(0, 512)], d_inT[:, bass.ts(0, 512)])
        for j in range(2):
            nc.gpsimd.dma_start(in8[:, j * SEQ:j * SEQ + 1024],
                                d_in8[:, j * SEQ:j * SEQ + 1024])
        nc.gpsimd.dma_start(wpv[:], d_wpv[:])
        nc.gpsimd.dma_start(wp[:], d_wp[:])
        nc.gpsimd.dma_start(iseq[:], d_iseq[:])
        for s in range(1, 4):
            sl = bass.ts(s, 512)
            nc.sync.dma_start(inT[:, sl], d_inT[:, sl])
        for j in range(2):
            nc.gpsimd.dma_start(in8[:, j * SEQ + 1024:(j + 1) * SEQ],
                                d_in8[:, j * SEQ + 1024:(j + 1) * SEQ])
        if use_bias:
            rb = per.tile([C_IN, 2], F32, tag="rb")
            bx = per.tile([128, F], F32, tag="bx")
            nc.sync.dma_start(rb[:], d_rb[:])
            nc.sync.dma_start(bx[:], d_bx[:])

        # Preload the exp table set (hidden in setup; first ACT of a new
        # table set costs ~2.7us on hw).
        warm = per.tile([128, 2], F32, tag="warm")
        nc.vector.memset(warm[:], 0.0)
        nc.scalar.activation(warm[:], warm[:],
                             mybir.ActivationFunctionType.Exp)

        # ---- Y8: y_q = m_qk^T x_q, channel-split fp8 [64,(ic,j,q)] ------
        # channel pairing c = p + 64j to match in8.
        Y8 = per.tile([64, N_IC * 2 * 512], FP8, tag="Y8")
        for ic in range(N_IC):
            p = ps_s.tile([64, 1024], F32, tag="ps_s",
                          name=f"py{ic}", padded_shape=[128, 1024])
            for j in range(2):
                nc.tensor.matmul(p[:, bass.ts(j, 512)],
                                 m2[:, j * 64:(j + 1) * 64],
                                 inT[:, bass.ts(ic, 512)],
                                 start=True, stop=True)
            dst = Y8[:, ic * 1024:(ic + 1) * 1024]
            if use_bias:
                for j in range(2):
                    eng[y8_eng].tensor_scalar_add(
                        dst[:, bass.ts(j, 512)], p[:, bass.ts(j, 512)],
                        rb[j * 64:(j + 1) * 64, 0:1])
            else:
                ecopy(y8_eng, dst, p[:])

        # ---- v8: V = X w_pvg, fp8 [128k, (jt, f)] ------------------------
        # Setup psums share the 3-deep scores rotation (ps_s tag); pc
        # slots hold only the C accumulators.
        ones8 = per.tile([128, 4], FP8, tag="ones8")
        nc.vector.memset(ones8[:], 1.0)
        x_sb = per.tile([128, (QROWS // 128) * F], F32, tag="x_sb")

        # ---- x_sb: residual x = X w_p (+ folded biases) ------------------
        for it2 in range(QROWS // 256):
            p = ps_s.tile([128, 512], F32, tag="ps_s", name=f"px{it2}",
                          padded_shape=[128, 1024])
            for j in range(2):
                it = 2 * it2 + j
                nc.tensor.matmul(p[:, bass.ts(j, F)],
                                 inT[:, bass.ts(it, 128)], wp[:],
                                 start=True, stop=True)
            dst = x_sb[:, bass.ts(it2, 512)]
            if use_bias:
                for j in range(2):
                    eng[x_engines[it2]].tensor_add(dst[:, bass.ts(j, F)],
                                                   p[:, bass.ts(j, F)], bx[:])
            else:
                ecopy(x_engines[it2], dst, p[:])

        # ---- attention ---------------------------------------------------
        in8_v = in8[:].rearrange("p (j k) -> p j k", j=2)       # [64,2,4096]
        Y8_v = Y8[:].rearrange("p (i j q) -> p i j q", i=N_IC, j=2)
        iseq_v = iseq[:].rearrange("p (t j c) -> p t j c", t=N_T2, j=2)
        ones_v = ones8[:].rearrange("p (j f) -> p j f", j=2)

        steps = [(ic, jt2) for ic in range(N_IC) for jt2 in range(N_T2)]
        sres = {}   # t2 -> scores psum tile
        eres = {}   # t2 -> list of per-isub fp8 views
        saved = {}  # ic -> {jt2 -> views} retained for the denominator pass
        pcs = {}    # ic -> (pc01, pc23) paired C accumulator tiles

        def emit_s(t2):
            ic, jt2 = steps[t2]
            ps = ps_s.tile([128, 1024], F32, tag="ps_s", name=f"ps{t2}")
            for u in range(2):
                jt = 2 * jt2 + u
                nc.tensor.matmul(ps[:, bass.ts(u, 512)],
                                 in8_v[:, :, jt * 128:(jt + 1) * 128],
                                 Y8_v[:, ic],
                                 start=True, stop=True, perf_mode=DR)
            sres[t2] = ps

        def emit_exp(t2):
            # One tile per engine so the stripes run truly concurrently;
            # each C matmul's [128,2,128] lhsT slice lives in one tile.
            ps = sres.pop(t2)
            na, nd, np_ = stripes[t2]
            ps3 = ps[:].rearrange("p (j q) -> p j q", j=2)
            views = [None] * 4
            tiles = []
            off = 0
            for which, n in (("act", na), ("dve", nd), ("pool", np_)):
                if n == 0:
                    continue
                w = n * 128
                if which == "act":
                    et = epool.tile([128, 2 * w], FP8, tag="e_a",
                                    name=f"ea{t2}", padded_shape=[128, 1024])
                    ev = et[:].rearrange("p (j q) -> p j q", j=2)
                    nc.scalar.activation(ev, ps3[:, :, off:off + w],
                                         mybir.ActivationFunctionType.Exp)
                else:
                    et = epool.tile([128, 2 * w], I8, tag="e_" + which[0],
                                    name=f"e{which[0]}{t2}",
                                    padded_shape=[128, 1024])
                    ev = et[:].rearrange("p (j q) -> p j q", j=2)
                    eng[which].tensor_scalar(ev, ps3[:, :, off:off + w],
                                             EXP_A, EXP_B,
                                             mybir.AluOpType.mult,
                                             mybir.AluOpType.add)
                    ev = et[:].bitcast(FP8).rearrange("p (j q) -> p j q", j=2)
                for i in range(n):
                    views[off // 128 + i] = ev[:, :, i * 128:(i + 1) * 128]
                tiles.append((ev, off, w))
                off += w
            eres[t2] = (views, tiles)

        def emit_c(t2):
            # factored attnV: C_x[ch, q] += X_k^T E over this key pair,
            # contracting 256 keys via DR; stationary = in_seq8 block.
            ic, jt2 = steps[t2]
            if jt2 == 0:
                pcs[ic] = ps_c.tile([128, 512], F32, tag="cx",
                                    name=f"cx{ic}", padded_shape=[128, 512])
            views, tiles = eres.pop(t2)
            saved.setdefault(ic, {})[jt2] = views
            cx = pcs[ic]
            for ev, off, w in tiles:
                nc.tensor.matmul(cx[:, off:off + w],
                                 iseq_v[:, jt2], ev,
                                 start=(jt2 == 0), stop=(jt2 == N_T2 - 1),
                                 perf_mode=DR)

        def emit_denoms(ic, pd, us, start, stop):
            sv = saved[ic]
            for u in us:
                for isub in range(4):
                    nc.tensor.matmul(pd[:, 2 * isub:2 * isub + 2],
                                     sv[u][isub], ones_v,
                                     start=(start and u == us[0]),
                                     stop=(stop and u == us[-1]),
                                     perf_mode=DR)

        def emit_epilogue(ic, pd=None):
            # denominators: tiny DR ones-matmuls over the ic's retained
            # e-tiles into one [128,8] psum (4 x [128,2] regions).  For
            # the last ic the accumulation was started early (no S-tile
            # allocations follow, so the rotation is unaffected).
            if pd is None:
                pd = ps_s.tile([128, 8], F32, tag="ps_s", name=f"pd{ic}",
                               padded_shape=[128, 1024])
                emit_denoms(ic, pd, list(range(N_T2)), True, True)
            saved.pop(ic)
            cx = pcs.pop(ic)
            cx8 = opool.tile([128, 512], mybir.dt.bfloat16, tag="cx8",
                             name=f"cx8_{ic}")
            ecopy("dve" if ic % 2 else "act", cx8[:], cx[:])
            proj = ps_s.tile([128, 1024], F32, tag="ps_s", name=f"pj{ic}",
                             padded_shape=[128, 1024])
            for isub in range(4):
                nc.tensor.matmul(proj[:, bass.ts(isub, F)],
                                 cx8[:, bass.ts(isub, 128)], wpv[:],
                                 start=True, stop=True)
            for isub in range(4):
                row = ic * 4 + isub
                src_c = proj[:, bass.ts(isub, F)]
                recip = opool.tile([128, 1], F32, tag="recip",
                                   name=f"recip{row}")
                nc.vector.reciprocal(recip[:], pd[:, 2 * isub:2 * isub + 1])
                o = opool.tile([128, F], F32, tag="o", name=f"o{row}")
                # last ic: shortest chain (recip -> affine -> DMA), both
                # out-queues, to minimize the drain tail
                ep = "dve" if ic == N_IC - 1 else ep_engines[isub]
                if ep == "dve":
                    nc.vector.affine_then_add(o[:], src_c,
                                              x_sb[:, bass.ts(row, F)],
                                              recip[:, 0:1], 0.0)
                else:
                    nc.scalar.mul(o[:], src_c, recip[:, 0:1])
                    nc.gpsimd.tensor_add(o[:], o[:],
                                         x_sb[:, bass.ts(row, F)])
                nc.sync.dma_start(d_out[row * 128:(row + 1) * 128, :], o[:])

        nsteps = len(steps)
        emit_s(0)
        emit_exp(0)
        for u in range(1, ahead):
            emit_s(u)
            emit_exp(u)
        pd_last = None
        for t2 in range(nsteps):
            emit_c(t2)
            ic, jt2 = steps[t2]
            if ic == N_IC - 1 and t2 + ahead >= nsteps:
                # past the final S allocation: accumulate denominators now
                if pd_last is None:
                    pd_last = ps_s.tile([128, 8], F32, tag="ps_s",
                                        name=f"pd{ic}",
                                        padded_shape=[128, 1024])
                    emit_denoms(ic, pd_last, list(range(jt2 + 1)),
                                True, False)
                else:
                    emit_denoms(ic, pd_last, [jt2], False,
                                jt2 == N_T2 - 1)
            if jt2 == N_T2 - 1:
                emit_epilogue(ic, pd_last if ic == N_IC - 1 else None)
            if t2 + ahead < nsteps:
                emit_s(t2 + ahead)
                emit_exp(t2 + ahead)

    nc.compile()
    return nc


_NC_CACHE = {}


def get_nc(**kw):
    key = tuple(sorted((k, str(v)) for k, v in kw.items()))
    if key not in _NC_CACHE:
        _NC_CACHE[key] = build_bass(**kw)
    return _NC_CACHE[key]


def _round_f32r(a):
    a = np.ascontiguousarray(np.asarray(a, np.float32))
    u = a.view(np.uint32)
    u = (u + np.uint32(1 << 10)) & np.uint32(0xFFFFF800)
    return u.view(np.float32)


def make_in_maps(inputs, W_proj, b_proj, W_q, b_q, W_k, b_k, W_v, b_v, gamma):
    import ml_dtypes
    NFP8 = ml_dtypes.float8_e4m3  # noqa: used for fp8 packing below
    f64 = np.float64
    Wp, Wq, Wk, Wv = [np.asarray(a, f64) for a in (W_proj, W_q, W_k, W_v)]
    bp, bq, bvv = [np.asarray(a, f64) for a in (b_proj, b_q, b_v)]
    g = float(np.asarray(gamma, f64).reshape(()))

    w_pq64, w_pk64 = Wp @ Wq, Wp @ Wk
    m2 = _round_f32r((w_pq64 @ w_pk64.T).astype(np.float32))
    w_pvb = (g * (Wp @ Wv)).astype(ml_dtypes.bfloat16)     # [128, 256]
    w_p = _round_f32r(np.asarray(W_proj, np.float32))
    bias_q64 = bp @ Wq + bq
    r_bias = np.zeros((128, 2), np.float32)
    r_bias[:, 0] = (w_pk64 @ bias_q64).astype(np.float32)
    bias_x = (np.asarray(b_proj, f64) + g * (bp @ Wv + bvv)).astype(np.float32)
    bias_x_bc = np.ascontiguousarray(np.broadcast_to(bias_x, (128, F)))
    use_bias = bool(np.abs(r_bias).max() > 0 or np.abs(bias_x).max() > 0)

    inp = np.asarray(inputs, np.float32).reshape(B, SEQ, C_IN)
    in_maps = []
    smax_est = 0.0
    for c in range(N_CORES):
        b, h = divmod(c, 2)
        rolled = np.roll(inp[b], -h * QROWS, axis=0) if h else inp[b]
        inT = _round_f32r(rolled[:QROWS].T)                     # [128, 2048]
        a8 = rolled.astype(NFP8)                                # [4096, 128]
        in8 = np.ascontiguousarray(
            a8.T.reshape(2, 64, SEQ).transpose(1, 0, 2).reshape(64, 2 * SEQ))
        in_seq8 = np.ascontiguousarray(
            a8.reshape(N_T2, 2, 128, C_IN).transpose(2, 0, 1, 3)
            .reshape(128, SEQ))
        if h == 0:
            # cheap sampled max-|score| estimate for the Schraudolph guard
            Ysm = (rolled[::16].astype(f64) @ m2.astype(f64))
            ssm = np.abs(Ysm @ rolled[::16].astype(f64).T).max()
            smax_est = max(smax_est, ssm)
        in_maps.append({
            "inT_r": inT, "in8": in8.view(np.uint8), "m2": m2,
            "in_seq8": in_seq8.view(np.uint8),
            "w_pvb": w_pvb.view(np.uint16), "w_p": w_p, "r_bias": r_bias,
            "bias_x_bc": bias_x_bc,
        })
    # Schraudolph needs |s|*8/ln2 + 56 within int8; stay well inside.
    safe = (smax_est * 2.5) * EXP_A + EXP_B < 120
    return in_maps, use_bias, safe


def kernel(inputs, W_proj, b_proj, W_q, b_q, W_k, b_k, W_v, b_v, gamma):
    in_maps, use_bias, safe = make_in_maps(
        inputs, W_proj, b_proj, W_q, b_q, W_k, b_k, W_v, b_v, gamma)
    kw = {"use_bias": use_bias}
    if not safe:
        # scores may overflow the int8 Schraudolph range: true exp only
        kw["stripes"] = [(4, 0, 0)] * 64
    nc = get_nc(**kw)
    res = run_bass_kernel_spmd(nc, in_maps, core_ids=list(range(N_CORES)))
    out = np.empty((B, SEQ, F), np.float32)
    for c in range(N_CORES):
        b, h = divmod(c, 2)
        out[b, h * QROWS:(h + 1) * QROWS] = res.results[c]["out"]
    return out.reshape(B, 64, 64, F)


if __name__ == "__main__":
    rng = np.random.default_rng(0)
    ins = {
        "inputs": rng.standard_normal((B, 64, 64, C_IN)).astype(np.float32),
        "W_proj": (rng.standard_normal((C_IN, F)) * 0.02).astype(np.float32),
        "b_proj": np.zeros(F, np.float32),
        "W_q": (rng.standard_normal((F, F)) * 0.02).astype(np.float32),
        "b_q": np.zeros(F, np.float32),
        "W_k": (rng.standard_normal((F, F)) * 0.02).astype(np.float32),
        "b_k": np.zeros(F, np.float32),
        "W_v": (rng.standard_normal((F, F)) * 0.02).astype(np.float32),
        "b_v": np.zeros(F, np.float32),
        "gamma": np.array([0.7], np.float32),
    }
    out = kernel(**ins)
    print("out", out.shape, out.dtype, float(np.abs(out).mean()))


# revision 38
# speedup vs baseline: 1.1138x; 1.1138x over previous
"""Trainium2 Bass kernel for nn_AttentionModule: full-sequence self-attention.

Reference computation (all fp32):
    x = inputs @ W_proj + b_proj            # [B,4096,256]   (B=4, N=4096)
    q,k,v = x@W_q+b_q, x@W_k+b_k, x@W_v+b_v
    attn = softmax(q @ k^T)                 # [B,4096,4096]
    out  = gamma * (attn @ v) + x

Sharding: 8 cores = 4 batches x 2 query-halves. Core c handles batch
b=c//2, query rows h*2048..h*2048+2048 (h=c%2); keys/values span the
full 4096 sequence of its batch (sequence rotated host-side; attention
is invariant under the joint key/value permutation).

Host-side algebra (exact up to fp reassociation):
    scores contract in channel space (rank <= C_IN=128):
        s_{q,k} = y_q . x_k + r . x_k,  y_q = m_qk^T x_q,
        m_qk = (Wp Wq)(Wp Wk)^T,  r = (Wp Wk)(bp Wq + b_q)
    attnV is factored through X:  E @ V = (E^T-contracted X) @ (g Wp Wv),
    so no V matrix is ever materialized on device; gamma and the value/
    residual biases fold into the projection weight / x-path bias.

Device program per core, fp8-e4m3 DoubleRow matmuls (0.5 cyc/row, 2x
the f32r rate; DR contracts 2x128 via [P,2,free] operand pairing):
    scores   S^T [128k x 512q] = DR(in8 [64,2,128k], Y8 [64,2,512q]);
             two key blocks per [128,1024] psum pair-tile (channel
             pairs c = p + 64j, split host-side).
    exp      each pair-tile's four 128-query sub-blocks split between
             ACT (true exp -> fp8) and DVE (exact-int8 Schraudolph:
             e = bitcast_e4m3(round(s*8/ln2 + 56)), piecewise-linear
             exp, <4% rel err - harmless since the attention context is
             ~0.2% of the residual x).  One e-tile PER ENGINE per step:
             the tile scheduler chains same-tile writers, which would
             serialize the stripes.  GPSIMD cannot read PSUM on hw, so
             it only gets SBUF-side epilogue adds.
    attnV    factored: C_x [128ch x 512q] += DR(in_seq8 [128k,2,128ch],
             e-tile [128k,2,w]) over 16 key-pair steps -- half the PE
             work of E@V and no V-conversion copies.  Per ic epilogue:
             C_x -> bf16, project with w_pvb [128,256] (4 matmuls into
             a scores-pool psum), denominators via a burst of tiny DR
             ones-matmuls over the ic's retained e-tiles (e-pool is 26
             deep so they survive), then out = proj*recip(denom) + x_sb
             (fused DVE affine_then_add or ACT-mul + Pool-add), DMA out.

f32r (tf32-like) matmuls for accuracy-critical paths: Y and the
residual x = X W_p (query columns only - inT is [128,2048]); f32r
inputs are pre-rounded host-side and DMA'd directly.  PSUM budget
(16KB/partition, bank-quantized slots): scores 3 bufs x 2 banks + C_x
2 bufs x 1 bank; setup/projection/denominator psums share the 3-deep
scores rotation.  Software pipelining: S-pairs emitted `ahead`=5 steps
before their C_x consumers; input DMA split across HWDGE + SWDGE
queues, critical chunks (m2, inT ic0, in8 head) first.

Cost-model exec time: 73.0us/core (original baseline 124.4us).
Measured on trn2 (8 cores): rel err 3.0e-04 vs fp32 jax reference.
"""

import numpy as np
from contextlib import ExitStack

import concourse.bass as bass
import concourse.tile as tile
from concourse import bacc, mybir
from concourse.bass_utils import run_bass_kernel_spmd

B, SEQ, C_IN, F = 4, 4096, 128, 256
N_CORES = 8
QROWS = SEQ // 2              # queries per core
N_IC = 4                      # 512-query chunks
N_T2 = 16                     # key-pair steps per ic (256 keys each)
F32, F32R, FP8 = mybir.dt.float32, mybir.dt.float32r, mybir.dt.float8e4
I8 = mybir.dt.int8
DR = mybir.MatmulPerfMode.DoubleRow
EXP_A = 8.0 / float(np.log(2.0))
EXP_B = 56.0


def default_stripes():
    # Per-step exp stripe split (act_cols, dve_cols) of the 512 query
    # columns (32-aligned).  Whole tiles alternate between ACT and DVE:
    # halves the per-instruction overhead count vs split stripes, and
    # the engines pipeline consecutive steps against each other.  The
    # factored C_x matmul consumes whole e-tiles, so only the tiny
    # denominator matmuls need tile-position-aligned segments.
    return [(512, 0) if t % 2 else (0, 512) for t in range(64)]


def default_vx_engines():
    # engines for the 16 v8 copies and 8 x copies (setup, front-loaded;
    # GPSIMD cannot read PSUM, so only act/dve are legal here)
    v = ["act", "dve"] * 8
    x = ["dve", "act"] * 4
    return v, x


def build_bass(stripes=None, s_bufs=3, e_bufs=26, use_bias=False,
               y8_eng="act", v_engines=None, x_engines=None,
               ep_engines=("dve", "actpool", "dve", "actpool"), ahead=5):
    if stripes is None:
        stripes = default_stripes()
    dv, dx = default_vx_engines()
    if v_engines is None:
        v_engines = dv
    if x_engines is None:
        x_engines = dx
    nc = bacc.Bacc("TRN2", target_bir_lowering=False, debug=False,
                   num_devices=N_CORES)
    d_inT = nc.dram_tensor("inT_r", [C_IN, QROWS], F32R, kind="ExternalInput").ap()
    d_in8 = nc.dram_tensor("in8", [64, 2 * SEQ], FP8, kind="ExternalInput").ap()
    d_m2 = nc.dram_tensor("m2", [C_IN, C_IN], F32R, kind="ExternalInput").ap()
    d_iseq = nc.dram_tensor("in_seq8", [128, SEQ], FP8, kind="ExternalInput").ap()
    d_wpv = nc.dram_tensor("w_pvb", [C_IN, F], mybir.dt.bfloat16,
                           kind="ExternalInput").ap()
    d_wp = nc.dram_tensor("w_p", [C_IN, F], F32R, kind="ExternalInput").ap()
    d_rb = nc.dram_tensor("r_bias", [C_IN, 2], F32, kind="ExternalInput").ap()
    d_bx = nc.dram_tensor("bias_x_bc", [128, F], F32, kind="ExternalInput").ap()
    d_out = nc.dram_tensor("out", [QROWS, F], F32, kind="ExternalOutput").ap()

    eng = {"act": nc.scalar, "dve": nc.vector, "pool": nc.gpsimd}

    def ecopy(which, dst, src_ap):
        if which == "act":
            nc.scalar.copy(dst, src_ap)
        else:
            eng[which].tensor_copy(dst, src_ap)

    with tile.TileContext(nc) as tc, ExitStack() as ctx:
        per = ctx.enter_context(tc.tile_pool(name="per", bufs=1))
        epool = ctx.enter_context(tc.tile_pool(name="epool", bufs=e_bufs))
        opool = ctx.enter_context(tc.tile_pool(name="opool", bufs=6))
        ps_s = ctx.enter_context(tc.tile_pool(name="ps_s", bufs=s_bufs,
                                              space="PSUM"))
        ps_c = ctx.enter_context(tc.tile_pool(name="ps_c", bufs=2,
                                              space="PSUM"))

        # ---- input DMA ---------------------------------------------------
        # Critical path to the first scores matmul: m2 + inT chunk 0
        # (-> Y8 ic0) and the first quarter of in8.  Order the sync queue
        # accordingly; everything else follows.
        m2 = per.tile([C_IN, C_IN], F32R, tag="m2")
        nc.sync.dma_start(m2[:], d_m2[:])
        inT = per.tile([C_IN, QROWS], F32R, tag="inT")
        in8 = per.tile([64, 2 * SEQ], FP8, tag="in8")
        iseq = per.tile([128, SEQ], FP8, tag="iseq")
        wpv = per.tile([C_IN, F], mybir.dt.bfloat16, tag="wpv")
        wp = per.tile([C_IN, F], F32R, tag="wp")
        # inT on the HWDGE queue; in8/weights in parallel on the SWDGE
        # queue (trigger cost lands on the otherwise-idle early Pool)
        nc.sync.dma_start(inT[:, bass.ts(0, 512)], d_inT[:, bass.ts(0, 512)])
        for j in range(2):
            nc.gpsimd.dma_start(in8[:, j * SEQ:j * SEQ + 1024],
                                d_in8[:, j * SEQ:j * SEQ + 1024])
        nc.gpsimd.dma_start(wpv[:], d_wpv[:])
        nc.gpsimd.dma_start(wp[:], d_wp[:])
        nc.gpsimd.dma_start(iseq[:], d_iseq[:])
        for s in range(1, 4):
            sl = bass.ts(s, 512)
            nc.sync.dma_start(inT[:, sl], d_inT[:, sl])
        for j in range(2):
            nc.gpsimd.dma_start(in8[:, j * SEQ + 1024:(j + 1) * SEQ],
                                d_in8[:, j * SEQ + 1024:(j + 1) * SEQ])
        if use_bias:
            rb = per.tile([C_IN, 2], F32, tag="rb")
            bx = per.tile([128, F], F32, tag="bx")
            nc.sync.dma_start(rb[:], d_rb[:])
            nc.sync.dma_start(bx[:], d_bx[:])

        # Preload the exp table set (hidden in setup; first ACT of a new
        # table set costs ~2.7us on hw).
        warm = per.tile([128, 2], F32, tag="warm")
        nc.vector.memset(warm[:], 0.0)
        nc.scalar.activation(warm[:], warm[:],
                             mybir.ActivationFunctionType.Exp)

        # ---- Y8: y_q = m_qk^T x_q, channel-split fp8 [64,(ic,j,q)] ------
        # channel pairing c = p + 64j to match in8.
        Y8 = per.tile([64, N_IC * 2 * 512], FP8, tag="Y8")
        for ic in range(N_IC):
            p = ps_s.tile([64, 1024], F32, tag="ps_s",
                          name=f"py{ic}", padded_shape=[128, 1024])
            for j in range(2):
                nc.tensor.matmul(p[:, bass.ts(j, 512)],
                                 m2[:, j * 64:(j + 1) * 64],
                                 inT[:, bass.ts(ic, 512)],
                                 start=True, stop=True)
            dst = Y8[:, ic * 1024:(ic + 1) * 1024]
            if use_bias:
                for j in range(2):
                    eng[y8_eng].tensor_scalar_add(
                        dst[:, bass.ts(j, 512)], p[:, bass.ts(j, 512)],
                        rb[j * 64:(j + 1) * 64, 0:1])
            else:
                ecopy(y8_eng, dst, p[:])

        # ---- v8: V = X w_pvg, fp8 [128k, (jt, f)] ------------------------
        # Setup psums share the 3-deep scores rotation (ps_s tag); pc
        # slots hold only the C accumulators.
        ones8 = per.tile([128, 4], FP8, tag="ones8")
        nc.vector.memset(ones8[:], 1.0)
        x_sb = per.tile([128, (QROWS // 128) * F], F32, tag="x_sb")

        # ---- x_sb: residual x = X w_p (+ folded biases) ------------------
        for it2 in range(QROWS // 256):
            p = ps_s.tile([128, 512], F32, tag="ps_s", name=f"px{it2}",
                          padded_shape=[128, 1024])
            for j in range(2):
                it = 2 * it2 + j
                nc.tensor.matmul(p[:, bass.ts(j, F)],
                                 inT[:, bass.ts(it, 128)], wp[:],
                                 start=True, stop=True)
            dst = x_sb[:, bass.ts(it2, 512)]
            if use_bias:
                for j in range(2):
                    eng[x_engines[it2]].tensor_add(dst[:, bass.ts(j, F)],
                                                   p[:, bass.ts(j, F)], bx[:])
            else:
                ecopy(x_engines[it2], dst, p[:])

        # ---- attention ---------------------------------------------------
        in8_v = in8[:].rearrange("p (j k) -> p j k", j=2)       # [64,2,4096]
        Y8_v = Y8[:].rearrange("p (i j q) -> p i j q", i=N_IC, j=2)
        iseq_v = iseq[:].rearrange("p (t j c) -> p t j c", t=N_T2, j=2)
        ones_v = ones8[:].rearrange("p (j f) -> p j f", j=2)

        steps = [(ic, jt2) for ic in range(N_IC) for jt2 in range(N_T2)]
        sres = {}   # t2 -> scores psum tile
        eres = {}   # t2 -> list of per-isub fp8 views
        saved = {}  # ic -> {jt2 -> views} retained for the denominator pass
        pcs = {}    # ic -> (pc01, pc23) paired C accumulator tiles

        def emit_s(t2):
            ic, jt2 = steps[t2]
            ps = ps_s.tile([128, 1024], F32, tag="ps_s", name=f"ps{t2}")
            for u in range(2):
                jt = 2 * jt2 + u
                nc.tensor.matmul(ps[:, bass.ts(u, 512)],
                                 in8_v[:, :, jt * 128:(jt + 1) * 128],
                                 Y8_v[:, ic],
                                 start=True, stop=True, perf_mode=DR)
            sres[t2] = ps

        def emit_exp(t2):
            # One tile per engine so the stripes run truly concurrently;
            # the factored C_x matmul takes each whole tile as its
            # moving operand, so stripe widths are free (32-aligned).
            ps = sres.pop(t2)
            a_cols, d_cols = stripes[t2]
            ps3 = ps[:].rearrange("p (j q) -> p j q", j=2)
            tiles = []
            off = 0
            for which, w in (("act", a_cols), ("dve", d_cols)):
                if w == 0:
                    continue
                if which == "act":
                    et = epool.tile([128, 2 * w], FP8, tag="e_a",
                                    name=f"ea{t2}", padded_shape=[128, 1024])
                    ev = et[:].rearrange("p (j q) -> p j q", j=2)
                    nc.scalar.activation(ev, ps3[:, :, off:off + w],
                                         mybir.ActivationFunctionType.Exp)
                else:
                    et = epool.tile([128, 2 * w], I8, tag="e_d",
                                    name=f"ed{t2}", padded_shape=[128, 1024])
                    ev = et[:].rearrange("p (j q) -> p j q", j=2)
                    nc.vector.tensor_scalar(ev, ps3[:, :, off:off + w],
                                            EXP_A, EXP_B,
                                            mybir.AluOpType.mult,
                                            mybir.AluOpType.add)
                    ev = et[:].bitcast(FP8).rearrange("p (j q) -> p j q", j=2)
                tiles.append((ev, off, w))
                off += w
            eres[t2] = tiles

        def emit_c(t2):
            # factored attnV: C_x[ch, q] += X_k^T E over this key pair,
            # contracting 256 keys via DR; stationary = in_seq8 block.
            ic, jt2 = steps[t2]
            if jt2 == 0:
                pcs[ic] = ps_c.tile([128, 512], F32, tag="cx",
                                    name=f"cx{ic}", padded_shape=[128, 512])
            tiles = eres.pop(t2)
            saved.setdefault(ic, {})[jt2] = tiles
            cx = pcs[ic]
            for ev, off, w in tiles:
                nc.tensor.matmul(cx[:, off:off + w],
                                 iseq_v[:, jt2], ev,
                                 start=(jt2 == 0), stop=(jt2 == N_T2 - 1),
                                 perf_mode=DR)

        def _dsegs(off, w):
            # decompose [off, off+w) into tile-position-aligned pieces
            # (col offsets {0,32,64,96} for 32-wide, {0,64} for 64, 0
            # for 128) within each 128-query sub-block
            segs = []
            qs, qe = off, off + w
            while qs < qe:
                rel = qs % 128
                n = min(qe - qs, 128 - rel)
                for size in (128, 64, 32):
                    if rel % size == 0 and size <= n:
                        segs.append((qs, size))
                        qs += size
                        break
                else:
                    raise ValueError((off, w))
            return segs

        def emit_denoms(ic, pd, us, start, stop):
            sv = saved[ic]
            for u in us:
                for ev, off, w in sv[u]:
                    for qs, size in _dsegs(off, w):
                        isub, rel = qs // 128, qs % 128
                        nc.tensor.matmul(
                            pd[rel:rel + size, 2 * isub:2 * isub + 2],
                            ev[:, :, qs - off:qs - off + size], ones_v,
                            start=(start and u == us[0]),
                            stop=(stop and u == us[-1]),
                            perf_mode=DR)

        def emit_epilogue(ic, pd=None):
            # denominators: tiny DR ones-matmuls over the ic's retained
            # e-tiles into one [128,8] psum (4 x [128,2] regions).  For
            # the last ic the accumulation was started early (no S-tile
            # allocations follow, so the rotation is unaffected).
            if pd is None:
                pd = ps_s.tile([128, 8], F32, tag="ps_s", name=f"pd{ic}",
                               padded_shape=[128, 1024])
                emit_denoms(ic, pd, list(range(N_T2)), True, True)
            saved.pop(ic)
            cx = pcs.pop(ic)
            cx8 = opool.tile([128, 512], mybir.dt.bfloat16, tag="cx8",
                             name=f"cx8_{ic}")
            ecopy("dve" if ic % 2 else "act", cx8[:], cx[:])
            proj = ps_s.tile([128, 1024], F32, tag="ps_s", name=f"pj{ic}",
                             padded_shape=[128, 1024])
            for isub in range(4):
                nc.tensor.matmul(proj[:, bass.ts(isub, F)],
                                 cx8[:, bass.ts(isub, 128)], wpv[:],
                                 start=True, stop=True)
            for isub in range(4):
                row = ic * 4 + isub
                src_c = proj[:, bass.ts(isub, F)]
                recip = opool.tile([128, 1], F32, tag="recip",
                                   name=f"recip{row}")
                nc.vector.reciprocal(recip[:], pd[:, 2 * isub:2 * isub + 1])
                o = opool.tile([128, F], F32, tag="o", name=f"o{row}")
                # last ic: shortest chain (recip -> affine -> DMA), both
                # out-queues, to minimize the drain tail
                ep = "dve" if ic == N_IC - 1 else ep_engines[isub]
                if ep == "dve":
                    nc.vector.affine_then_add(o[:], src_c,
                                              x_sb[:, bass.ts(row, F)],
                                              recip[:, 0:1], 0.0)
                else:
                    nc.scalar.mul(o[:], src_c, recip[:, 0:1])
                    nc.gpsimd.tensor_add(o[:], o[:],
                                         x_sb[:, bass.ts(row, F)])
                nc.sync.dma_start(d_out[row * 128:(row + 1) * 128, :], o[:])

        nsteps = len(steps)
        emit_s(0)
        emit_exp(0)
        for u in range(1, ahead):
            emit_s(u)
            emit_exp(u)
        pd_last = None
        for t2 in range(nsteps):
            emit_c(t2)
            ic, jt2 = steps[t2]
            if ic == N_IC - 1 and t2 + ahead >= nsteps:
                # past the final S allocation: accumulate denominators now
                if pd_last is None:
                    pd_last = ps_s.tile([128, 8], F32, tag="ps_s",
                                        name=f"pd{ic}",
                                        padded_shape=[128, 1024])
                    emit_denoms(ic, pd_last, list(range(jt2 + 1)),
                                True, False)
                else:
                    emit_denoms(ic, pd_last, [jt2], False,
                                jt2 == N_T2 - 1)
            if jt2 == N_T2 - 1:
                emit_epilogue(ic, pd_last if ic == N_IC - 1 else None)
            if t2 + ahead < nsteps:
                emit_s(t2 + ahead)
                emit_exp(t2 + ahead)

    nc.compile()
    return nc


_NC_CACHE = {}


def get_nc(**kw):
    key = tuple(sorted((k, str(v)) for k, v in kw.items()))
    if key not in _NC_CACHE:
        _NC_CACHE[key] = build_bass(**kw)
    return _NC_CACHE[key]


def _round_f32r(a):
    a = np.ascontiguousarray(np.asarray(a, np.float32))
    u = a.view(np.uint32)
    u = (u + np.uint32(1 << 10)) & np.uint32(0xFFFFF800)
    return u.view(np.float32)


def make_in_maps(inputs, W_proj, b_proj, W_q, b_q, W_k, b_k, W_v, b_v, gamma):
    import ml_dtypes
    NFP8 = ml_dtypes.float8_e4m3  # noqa: used for fp8 packing below
    f64 = np.float64
    Wp, Wq, Wk, Wv = [np.asarray(a, f64) for a in (W_proj, W_q, W_k, W_v)]
    bp, bq, bvv = [np.asarray(a, f64) for a in (b_proj, b_q, b_v)]
    g = float(np.asarray(gamma, f64).reshape(()))

    w_pq64, w_pk64 = Wp @ Wq, Wp @ Wk
    m2 = _round_f32r((w_pq64 @ w_pk64.T).astype(np.float32))
    w_pvb = (g * (Wp @ Wv)).astype(ml_dtypes.bfloat16)     # [128, 256]
    w_p = _round_f32r(np.asarray(W_proj, np.float32))
    bias_q64 = bp @ Wq + bq
    r_bias = np.zeros((128, 2), np.float32)
    r_bias[:, 0] = (w_pk64 @ bias_q64).astype(np.float32)
    bias_x = (np.asarray(b_proj, f64) + g * (bp @ Wv + bvv)).astype(np.float32)
    bias_x_bc = np.ascontiguousarray(np.broadcast_to(bias_x, (128, F)))
    use_bias = bool(np.abs(r_bias).max() > 0 or np.abs(bias_x).max() > 0)

    inp = np.asarray(inputs, np.float32).reshape(B, SEQ, C_IN)
    in_maps = []
    smax_est = 0.0
    for c in range(N_CORES):
        b, h = divmod(c, 2)
        rolled = np.roll(inp[b], -h * QROWS, axis=0) if h else inp[b]
        inT = _round_f32r(rolled[:QROWS].T)                     # [128, 2048]
        a8 = rolled.astype(NFP8)                                # [4096, 128]
        in8 = np.ascontiguousarray(
            a8.T.reshape(2, 64, SEQ).transpose(1, 0, 2).reshape(64, 2 * SEQ))
        in_seq8 = np.ascontiguousarray(
            a8.reshape(N_T2, 2, 128, C_IN).transpose(2, 0, 1, 3)
            .reshape(128, SEQ))
        if h == 0:
            # cheap sampled max-|score| estimate for the Schraudolph guard
            Ysm = (rolled[::16].astype(f64) @ m2.astype(f64))
            ssm = np.abs(Ysm @ rolled[::16].astype(f64).T).max()
            smax_est = max(smax_est, ssm)
        in_maps.append({
            "inT_r": inT, "in8": in8.view(np.uint8), "m2": m2,
            "in_seq8": in_seq8.view(np.uint8),
            "w_pvb": w_pvb.view(np.uint16), "w_p": w_p, "r_bias": r_bias,
            "bias_x_bc": bias_x_bc,
        })
    # Schraudolph needs |s|*8/ln2 + 56 within int8; stay well inside.
    safe = (smax_est * 2.5) * EXP_A + EXP_B < 120
    return in_maps, use_bias, safe


def kernel(inputs, W_proj, b_proj, W_q, b_q, W_k, b_k, W_v, b_v, gamma):
    in_maps, use_bias, safe = make_in_maps(
        inputs, W_proj, b_proj, W_q, b_q, W_k, b_k, W_v, b_v, gamma)
    kw = {"use_bias": use_bias}
    if not safe:
        # scores may overflow the int8 Schraudolph range: true exp only
        kw["stripes"] = [(512, 0)] * 64
    nc = get_nc(**kw)
    res = run_bass_kernel_spmd(nc, in_maps, core_ids=list(range(N_CORES)))
    out = np.empty((B, SEQ, F), np.float32)
    for c in range(N_CORES):
        b, h = divmod(c, 2)
        out[b, h * QROWS:(h + 1) * QROWS] = res.results[c]["out"]
    return out.reshape(B, 64, 64, F)


if __name__ == "__main__":
    rng = np.random.default_rng(0)
    ins = {
        "inputs": rng.standard_normal((B, 64, 64, C_IN)).astype(np.float32),
        "W_proj": (rng.standard_normal((C_IN, F)) * 0.02).astype(np.float32),
        "b_proj": np.zeros(F, np.float32),
        "W_q": (rng.standard_normal((F, F)) * 0.02).astype(np.float32),
        "b_q": np.zeros(F, np.float32),
        "W_k": (rng.standard_normal((F, F)) * 0.02).astype(np.float32),
        "b_k": np.zeros(F, np.float32),
        "W_v": (rng.standard_normal((F, F)) * 0.02).astype(np.float32),
        "b_v": np.zeros(F, np.float32),
        "gamma": np.array([0.7], np.float32),
    }
    out = kernel(**ins)
    print("out", out.shape, out.dtype, float(np.abs(out).mean()))
